# revision 1
# baseline (speedup 1.0000x reference)
"""Masked multi-head attention on 8 TRN2 NeuronCores.

Sharding: core = (batch b, head-group hg). Each core computes the attention
output for one batch element and 4 of the 8 heads (a 256-wide column slice
of E). Rows with mask==0 are dropped host-side before the kernel runs:
masked queries produce all-zero output rows, and masked keys are excluded
from the softmax, so the kernel only processes the ~half of S that is live
(gathered and padded to a multiple of 128).

Input is packed host-side into one bf16/fp32r blob per core and DMA'd in
progressive chunks over both HWDGE rings so the v/k projections start
while later xT groups are still in flight.

Per-core on-chip pipeline (scores in fp32r, PV in bf16, fp32 PSUM):
  qT/kT = W.T @ xT         (E-cols on partitions, S free)
  v     = xT.T @ Wv        (S on partitions, DH free) + ones column
  sT    = kT_chunk.T @ qT  (keys on partitions, queries free)
  att   = exp(sT/8 + pad_bias)                  [ACT, bias masks pad keys]
  hT   += v_aug.T @ att    (accumulates h' and the softmax denominator)
  out   = hT (+den row) DMA'd per head; the host transposes and divides
"""

import os

import numpy as np

import concourse.bacc as bacc
import concourse.tile as tile
from concourse import mybir
from concourse.bass_utils import run_bass_kernel_spmd

BF = mybir.dt.bfloat16
F32 = mybir.dt.float32
MMDT = mybir.dt.float32r  # fp32 storage, full-rate single-pass PE mode

B, S, F, E, H = 4, 2048, 512, 512, 8
DH = 64
NCORES = 8
HPC = 4            # heads per core
CPC = HPC * DH     # output columns per core

LAST_RESULT = None  # BassKernelResults of the most recent run (for test harness)


def _qchunks(SPL):
    # 512-wide chunks plus remainder; boundaries stay 128-aligned for the
    # kc-major xT layout. SPL must be even (fp32r 128-row matmuls).
    out, off = [], 0
    while off < SPL:
        ln = min(512, SPL - off)
        out.append((off, ln))
        off += ln
    return out


def _offsets(SP):
    # W stored v|k|q so the v projection can start earliest; xT stored
    # kc-major so progressive DMAs unlock v-projection chunks as they land
    WV_OFF = 0
    WK_OFF = 1024
    WQ_OFF = 2048
    BK_OFF = 3072
    BV_OFF = BK_OFF + 256
    ONES_OFF = BK_OFF + 512
    ONES2_OFF = ONES_OFF + SP      # [128, HPC] of ones (v_aug denominator cols)
    XT_OFF = ONES2_OFF + HPC
    ETE_OFF = XT_OFF + 4 * SP
    COLS = ETE_OFF + 2 * SP
    return WV_OFF, WK_OFF, WQ_OFF, XT_OFF, ETE_OFF, BK_OFF, BV_OFF, ONES_OFF, ONES2_OFF, COLS


def _build(SP, loop_reps=None, abl="full", SPL=None, pairq=True, has_bias=True):
    if SPL is None:
        SPL = SP
    NKC = SP // 128
    (WV_OFF, WK_OFF, WQ_OFF, XT_OFF, ETE_OFF, BK_OFF, BV_OFF, ONES_OFF,
     ONES2_OFF, COLS) = _offsets(SP)

    nc = bacc.Bacc()
    blob = nc.declare_dram_parameter("blob", [128, COLS], MMDT, isOutput=False)
    miscf = nc.declare_dram_parameter("miscf", [128, NKC + 65], F32, isOutput=False)
    outp = nc.declare_dram_parameter("out", [HPC, 65, SP], F32, isOutput=True)

    with tile.TileContext(nc) as tc:
        with (
            tc.tile_pool(name="sing", bufs=1) as sing,
            tc.tile_pool(name="hsb", bufs=3) as hsb_pool,
            tc.tile_pool(name="attp", bufs=6) as attp,
            tc.tile_pool(name="ps", bufs=2, space="PSUM") as ps,
        ):
            def _body():
                _emit(nc, SP, SPL, NKC, WV_OFF, WK_OFF, WQ_OFF, XT_OFF, ETE_OFF,
                      BK_OFF, BV_OFF, ONES_OFF, ONES2_OFF, COLS, blob, miscf, outp,
                      sing, hsb_pool, attp, ps, abl, pairq, has_bias)

            if loop_reps is None:
                _body()
            else:
                with tc.For_i(0, loop_reps, 1):
                    _body()
    nc.compile()
    return nc


def _xt_moving(bsb, XT_OFF, SP, f, qoff, qlen):
    """Moving-operand APs over the kc-major xT layout for q range [qoff, qoff+qlen)."""
    view = bsb[:, XT_OFF:XT_OFF + 4 * SP].rearrange("p (kc f c) -> p kc f c", f=4, c=128)
    out = []
    kc0, nfull, rem = qoff // 128, qlen // 128, qlen % 128
    if nfull:
        out.append((0, nfull * 128, view[:, kc0:kc0 + nfull, f, :]))
    if rem:
        out.append((nfull * 128, rem, view[:, kc0 + nfull, f, :rem]))
    return out


def _emit(nc, SP, SPL, NKC, WV_OFF, WK_OFF, WQ_OFF, XT_OFF, ETE_OFF, BK_OFF,
          BV_OFF, ONES_OFF, ONES2_OFF, COLS, blob, miscf, outp, sing, hsb_pool,
          attp, ps, abl="full", pairq=True, has_bias=True):
    QCH = _qchunks(SPL)
    NKCL = (SPL + 127) // 128
    # pairs of consecutive q chunks share one 2-bank psum tile / one exp op
    if pairq:
        PAIRS = [QCH[i:i + 2] for i in range(0, len(QCH), 2)]
    else:
        PAIRS = [[c] for c in QCH]
    PW = max(len(p) for p in PAIRS)
    SBUFS = 3 if PW == 2 else 4

    bsb = sing.tile([128, COLS], MMDT)
    msb = sing.tile([128, NKC + 65], F32)
    KG = [(0, min(4, NKC))]
    while KG[-1][1] < NKC:
        KG.append((KG[-1][1], min(KG[-1][1] + 4, NKC)))
    nc.sync.dma_start(out=bsb[:, :WK_OFF], in_=blob[:, :WK_OFF])
    nc.scalar.dma_start(out=msb, in_=miscf[:, :])
    nc.scalar.dma_start(out=bsb[:, WK_OFF:XT_OFF], in_=blob[:, WK_OFF:XT_OFF])
    for gi, (k0, k1) in enumerate(KG):
        eng = nc.sync if gi % 2 == 0 else nc.scalar
        c0, c1 = XT_OFF + k0 * 512, XT_OFF + k1 * 512
        eng.dma_start(out=bsb[:, c0:c1], in_=blob[:, c0:c1])
    nc.scalar.dma_start(out=bsb[:, ETE_OFF:], in_=blob[:, ETE_OFF:])

    qk = sing.tile([128, 4, SP], BF if abl == "bf16sc" else MMDT)  # qT 0-1, kT 2-3
    vall = sing.tile([128, NKC, 65 * HPC], BF)
    scr_d = sing.tile([1, 1], MMDT)
    scr_a = sing.tile([1, 1], F32)

    ones_row = bsb[0:1, ONES_OFF:ONES_OFF + SP]

    # Engine preambles: each engine observes the input DMA lanes via a cheap
    # op so no later instruction needs two fresh semaphore waits (hardware
    # allows one sync wait per instruction; extras cost event-sem splits).
    nc.vector.tensor_copy(scr_d, bsb[0:1, 0:1])
    nc.vector.tensor_copy(scr_d, bsb[0:1, XT_OFF:XT_OFF + 1])
    nc.vector.tensor_copy(scr_d, bsb[0:1, ETE_OFF:ETE_OFF + 1])
    nc.scalar.copy(scr_a, msb[0:1, 0:1])

    # ones columns of v_aug, early so PE's DVE clock covers them
    ones2 = bsb[:, ONES2_OFF:ONES2_OFF + HPC]
    for kc in range(NKC):
        va = vall[:, kc, :].rearrange("p (h c) -> p h c", c=65)
        nc.vector.tensor_copy(va[:, :, 64:65], ones2.rearrange("p (h c) -> p h c", c=1))

    if abl == "dmas":
        return

    # ---- projections, interleaved with the progressive xT DMA groups:
    # group g unlocks v[kc in g] and the k/q chunks whose kc range lies in
    # groups <= g. v first, then k; q last (waits on the ete DMA anyway).
    def v_proj(kc):
        pv = ps.tile([128, PW, 512], F32, tag="s2", bufs=SBUFS, name="pv")
        if has_bias:
            nc.tensor.matmul(pv[:, 0, :256], ones_row[:, 0:128], bsb[0:1, BV_OFF:BV_OFF + 256],
                             start=True, stop=False)
        for f in range(4):
            base = XT_OFF + (kc * 4 + f) * 128
            lhsT = bsb[:, base:base + 128]
            rhs = bsb[:, WV_OFF + f * 256:WV_OFF + (f + 1) * 256]
            nc.tensor.matmul(pv[:, 0, :256], lhsT, rhs,
                             start=(f == 0 and not has_bias), stop=(f == 3))
        va = vall[:, kc, :].rearrange("p (h c) -> p h c", c=65)
        nc.vector.tensor_copy(va[:, :, 0:64], pv[:, 0, :256].rearrange("p (h c) -> p h c", c=64))

    def kq_proj(cc, pair):
        p = ps.tile([128, PW, 512], F32, tag="s2", bufs=SBUFS, name="p")
        for j, (qoff, qlen) in enumerate(pair):
            if cc >= 2 and has_bias:  # k: rank-1 bias init (bk x ones)
                bksl = bsb[0:1, BK_OFF + (cc - 2) * 128:BK_OFF + (cc - 1) * 128]
                nc.tensor.matmul(p[:, j, :qlen], bksl, ones_row[:, qoff:qoff + qlen],
                                 start=True, stop=False)
            nparts = len(_xt_moving(bsb, XT_OFF, SP, 0, qoff, qlen))
            for pi in range(nparts):
                for f in range(4):
                    if cc < 2:
                        woff = WQ_OFF + f * 256 + cc * 128
                    else:
                        woff = WK_OFF + f * 256 + (cc - 2) * 128
                    lhsT = bsb[:, woff:woff + 128]
                    loff, llen, ap = _xt_moving(bsb, XT_OFF, SP, f, qoff, qlen)[pi]
                    nc.tensor.matmul(p[:, j, loff:loff + llen], lhsT, ap,
                                     start=(f == 0 and (cc < 2 or not has_bias)),
                                     stop=(f == 3))
        qoff0 = pair[0][0]
        width = (512 + pair[1][1]) if len(pair) == 2 else pair[0][1]
        pview = p[:].rearrange("p a b -> p (a b)")[:, :width]
        if cc < 2:  # q: add etype_emb (includes bq)
            ete_sl = bsb[:, ETE_OFF + cc * SP + qoff0:ETE_OFF + cc * SP + qoff0 + width]
            nc.vector.tensor_add(qk[:, cc, qoff0:qoff0 + width], pview, ete_sl)
        else:
            nc.vector.tensor_copy(qk[:, cc, qoff0:qoff0 + width], pview)

    QCH_K = _qchunks(SP)
    if pairq:
        PAIRS_K = [QCH_K[i:i + 2] for i in range(0, len(QCH_K), 2)]
    else:
        PAIRS_K = [[c] for c in QCH_K]

    def pair_group(pair):  # last xT group this pair's kc range touches
        qoff, qlen = pair[-1]
        return ((qoff + qlen - 1) // 128) // 4

    for gi, (k0, k1) in enumerate(KG):
        for kc in range(k0, k1):
            v_proj(kc)
        for cc in (2, 3):
            for pair in PAIRS_K:
                if pair_group(pair) == gi:
                    kq_proj(cc, pair)
    for cc in (0, 1):
        for pair in PAIRS:
            kq_proj(cc, pair)

    if abl == "proj":
        return

    # ---- attention, software-pipelined: the scores matmuls for step i+1 are
    # emitted before step i's PV matmuls, so the in-order PE never sits
    # behind the exp running on ACT.
    def scores_mm(step, sp_tile):
        h, ip, kc = step
        cbase = (h % 2) * 64
        for j, (qoff, qlen) in enumerate(PAIRS[ip]):
            lhsT = qk[cbase:cbase + 64, 2 + h // 2, kc * 128:(kc + 1) * 128]
            rhs = qk[cbase:cbase + 64, h // 2, qoff:qoff + qlen]
            nc.tensor.matmul(sp_tile[:, j, :qlen], lhsT, rhs, start=True, stop=True)

    steps = [(h, ip, kc)
             for h in range(HPC) for ip in range(len(PAIRS)) for kc in range(NKC)]
    hts = None
    hp = {}
    DEPTH = 2
    sp_q = []
    for d in range(min(DEPTH, len(steps))):
        t = ps.tile([128, PW, 512], F32, tag="s2", bufs=SBUFS, name="sp_t")
        scores_mm(steps[d], t)
        sp_q.append(t)
    for i, step in enumerate(steps):
        h, ip, kc = step
        pair = PAIRS[ip]
        width = (512 + pair[1][1]) if len(pair) == 2 else pair[0][1]
        sp_cur = sp_q.pop(0)
        if i + DEPTH < len(steps):
            sp_next = ps.tile([128, PW, 512], F32, tag="s2", bufs=SBUFS, name="sp_t")
            scores_mm(steps[i + DEPTH], sp_next)
            sp_q.append(sp_next)
        att = attp.tile([128, PW, 512], BF, tag="att")
        nc.scalar.activation(att[:].rearrange("p a b -> p (a b)")[:, :width],
                             sp_cur[:].rearrange("p a b -> p (a b)")[:, :width],
                             mybir.ActivationFunctionType.Exp,
                             bias=msb[:, kc:kc + 1], scale=0.125)
        if abl != "nopv":
            if kc == 0 and ip == 0:
                hts = hsb_pool.tile([65, NKCL * 128], F32, tag="hts")
            for j, (qoff, qlen) in enumerate(pair):
                if kc == 0:
                    hp[j] = ps.tile([65, 512], F32, tag="h", name="hp")
                nc.tensor.matmul(hp[j][:, :qlen], vall[:, kc, h * 65:(h + 1) * 65],
                                 att[:, j, :qlen], start=(kc == 0), stop=(kc == NKC - 1))
            if kc == NKC - 1:
                for j, (qoff, qlen) in enumerate(pair):
                    nc.vector.tensor_copy(hts[:, qoff:qoff + qlen], hp[j][:, :qlen])
                if ip == len(PAIRS) - 1:  # head done: ship hT (+den row); the
                    # host does the [64, q] -> [q, 64] transpose and divide
                    nc.sync.dma_start(out=outp[h, :, :NKCL * 128], in_=hts[:])




def _prep_core(core, SP, x, etype_emb, mask, Wq, bq, Wk, bk, Wv, bv):
    NKC = SP // 128
    (WV_OFF, WK_OFF, WQ_OFF, XT_OFF, ETE_OFF, BK_OFF, BV_OFF, ONES_OFF,
     ONES2_OFF, COLS) = _offsets(SP)
    b, hg = core // 2, core % 2
    c0 = hg * CPC
    idx = np.where(mask[b] == 1)[0]
    Su = len(idx)

    blob = np.zeros((128, COLS), np.float32)
    xs = np.zeros((SP, F), np.float32)
    xs[:Su] = x[b][idx]
    xT = xs.T
    xtb = xT.reshape(4, 128, NKC, 128).transpose(1, 2, 0, 3).reshape(128, NKC * 512)
    blob[:, XT_OFF:XT_OFF + 4 * SP] = xtb
    for f in range(4):
        blob[:, WV_OFF + f * 256:WV_OFF + (f + 1) * 256] = Wv[f * 128:(f + 1) * 128, c0:c0 + CPC]
        blob[:, WK_OFF + f * 256:WK_OFF + (f + 1) * 256] = Wk[f * 128:(f + 1) * 128, c0:c0 + CPC]
        blob[:, WQ_OFF + f * 256:WQ_OFF + (f + 1) * 256] = Wq[f * 128:(f + 1) * 128, c0:c0 + CPC]
    et = np.zeros((SP, CPC), np.float32)
    et[:Su] = etype_emb[b][idx][:, c0:c0 + CPC] + bq[c0:c0 + CPC]
    etT = et.T
    blob[:, ETE_OFF:ETE_OFF + SP] = etT[:128]
    blob[:, ETE_OFF + SP:ETE_OFF + 2 * SP] = etT[128:]
    blob[0, BK_OFF:BK_OFF + CPC] = bk[c0:c0 + CPC]
    blob[0, BV_OFF:BV_OFF + CPC] = bv[c0:c0 + CPC]
    blob[0, ONES_OFF:ONES_OFF + SP] = 1.0
    blob[:, ONES2_OFF:ONES2_OFF + HPC] = 1.0

    miscf = np.zeros((128, NKC + 65), np.float32)
    pos = np.arange(128)[:, None] + 128 * np.arange(NKC)[None, :]
    miscf[:, :NKC] = np.where(pos < Su, 0.0, -30000.0)
    miscf[:65, NKC:NKC + 65] = np.eye(65, dtype=np.float32)

    return {"blob": blob, "miscf": miscf}, idx


def kernel(x, etype_emb, mask, Wq, bq, Wk, bk, Wv, bv):
    global LAST_RESULT
    x = np.asarray(x, np.float32)
    etype_emb = np.asarray(etype_emb, np.float32)
    mask = np.asarray(mask)
    Wq, bq = np.asarray(Wq, np.float32), np.asarray(bq, np.float32)
    Wk, bk = np.asarray(Wk, np.float32), np.asarray(bk, np.float32)
    Wv, bv = np.asarray(Wv, np.float32), np.asarray(bv, np.float32)

    counts = [int((mask[b] == 1).sum()) for b in range(B)]
    SPL = max(2, max(counts))
    SPL += SPL % 2  # fp32r matmuls with 128 contraction rows need even N
    SP = max(128, ((SPL + 127) // 128) * 128)

    has_bias = bool(np.any(bk) or np.any(bv))
    nc = _build(SP, SPL=SPL, has_bias=has_bias)
    in_maps, idxs = [], []
    for core in range(NCORES):
        m, idx = _prep_core(core, SP, x, etype_emb, mask, Wq, bq, Wk, bk, Wv, bv)
        in_maps.append(m)
        idxs.append(idx)

    # The NTFF trace path needs antenv.axon_hooks, which this container does
    # not ship; make sure a stray BASS_TRACE=1 cannot route us into it.
    os.environ.setdefault("BASS_NEVER_TRACE", "1")
    res = run_bass_kernel_spmd(nc, in_maps, list(range(NCORES)))
    LAST_RESULT = res

    out = np.zeros((B, S, E), np.float32)
    for core in range(NCORES):
        b, hg = core // 2, core % 2
        idx = idxs[core]
        if not len(idx):
            continue
        shard = res.results[core]["out"]  # [HPC, 65, SP]: hT rows + denominator
        for h in range(HPC):
            num = shard[h, :64, :len(idx)]
            den = shard[h, 64, :len(idx)]
            out[b][idx, hg * CPC + h * 64:hg * CPC + (h + 1) * 64] = (num / den).T
    return out



# revision 7
# speedup vs baseline: 1.3264x; 1.3264x over previous
"""Masked multi-head attention on 8 TRN2 NeuronCores.

Sharding: core = (batch b, head-group hg). Each core computes the attention
output for one batch element and 4 of the 8 heads (a 256-wide column slice
of E). Rows with mask==0 are dropped host-side before the kernel runs:
masked queries produce all-zero output rows, and masked keys are excluded
from the softmax, so the kernel only processes the ~half of S that is live
(gathered and padded to a multiple of 128).

Input is packed host-side into one bf16/fp32r blob per core and DMA'd in
progressive chunks over both HWDGE rings so the v/k projections start
while later xT groups are still in flight.

Per-core on-chip pipeline (scores in fp32r, PV in bf16, fp32 PSUM):
  qT/kT = W.T @ xT         (E-cols on partitions, S free)
  v     = xT.T @ Wv        (S on partitions, DH free) + ones column
  sT    = kT_chunk.T @ qT  (keys on partitions, queries free)
  att   = exp(sT/8 + pad_bias)                  [ACT, bias masks pad keys]
  hT   += v_aug.T @ att    (accumulates h' and the softmax denominator)
  out   = hT (+den row) DMA'd per head; the host transposes and divides
"""

import os

import ml_dtypes
import numpy as np

import concourse.bacc as bacc
import concourse.tile as tile
from concourse import mybir
from concourse.bass_utils import run_bass_kernel_spmd

BF = mybir.dt.bfloat16
F32 = mybir.dt.float32
MMDT = mybir.dt.bfloat16  # all matmul operands in bf16 (full-rate, half DMA)

B, S, F, E, H = 4, 2048, 512, 512, 8
DH = 64
NCORES = 8
HPC = 4            # heads per core
CPC = HPC * DH     # output columns per core

LAST_RESULT = None  # BassKernelResults of the most recent run (for test harness)


def _qchunks(SPL):
    # 512-wide chunks plus remainder; boundaries stay 128-aligned for the
    # kc-major xT layout. SPL must be even (fp32r 128-row matmuls).
    out, off = [], 0
    while off < SPL:
        ln = min(512, SPL - off)
        out.append((off, ln))
        off += ln
    return out


def _offsets(SP):
    # W stored v|k|q so the v projection can start earliest; xT stored
    # kc-major so progressive DMAs unlock v-projection chunks as they land
    WV_OFF = 0
    WK_OFF = 1024
    WQ_OFF = 2048
    BK_OFF = 3072
    BV_OFF = BK_OFF + 256
    ONES_OFF = BK_OFF + 512
    ONES2_OFF = ONES_OFF + SP      # [128, HPC] of ones (v_aug denominator cols)
    XT_OFF = ONES2_OFF + HPC
    ETE_OFF = XT_OFF + 4 * SP
    COLS = ETE_OFF + 2 * SP
    return WV_OFF, WK_OFF, WQ_OFF, XT_OFF, ETE_OFF, BK_OFF, BV_OFF, ONES_OFF, ONES2_OFF, COLS


def _build(SP, loop_reps=None, abl="full", SPL=None, pairq=True, has_bias=True):
    if SPL is None:
        SPL = SP
    NKC = SP // 128
    (WV_OFF, WK_OFF, WQ_OFF, XT_OFF, ETE_OFF, BK_OFF, BV_OFF, ONES_OFF,
     ONES2_OFF, COLS) = _offsets(SP)

    nc = bacc.Bacc()
    blob = nc.declare_dram_parameter("blob", [128, COLS], MMDT, isOutput=False)
    miscf = nc.declare_dram_parameter("miscf", [128, NKC + 65], F32, isOutput=False)
    outp = nc.declare_dram_parameter("out", [HPC, 65, SP], F32, isOutput=True)

    with tile.TileContext(nc) as tc:
        with (
            tc.tile_pool(name="sing", bufs=1) as sing,
            tc.tile_pool(name="hsb", bufs=3) as hsb_pool,
            tc.tile_pool(name="attp", bufs=6) as attp,
            tc.tile_pool(name="ps", bufs=2, space="PSUM") as ps,
        ):
            def _body():
                _emit(nc, SP, SPL, NKC, WV_OFF, WK_OFF, WQ_OFF, XT_OFF, ETE_OFF,
                      BK_OFF, BV_OFF, ONES_OFF, ONES2_OFF, COLS, blob, miscf, outp,
                      sing, hsb_pool, attp, ps, abl, pairq, has_bias)

            if loop_reps is None:
                _body()
            else:
                with tc.For_i(0, loop_reps, 1):
                    _body()
    nc.compile()
    return nc


def _xt_moving(bsb, XT_OFF, SP, f, qoff, qlen):
    """Moving-operand APs over the kc-major xT layout for q range [qoff, qoff+qlen)."""
    view = bsb[:, XT_OFF:XT_OFF + 4 * SP].rearrange("p (kc f c) -> p kc f c", f=4, c=128)
    out = []
    kc0, nfull, rem = qoff // 128, qlen // 128, qlen % 128
    if nfull:
        out.append((0, nfull * 128, view[:, kc0:kc0 + nfull, f, :]))
    if rem:
        out.append((nfull * 128, rem, view[:, kc0 + nfull, f, :rem]))
    return out


def _emit(nc, SP, SPL, NKC, WV_OFF, WK_OFF, WQ_OFF, XT_OFF, ETE_OFF, BK_OFF,
          BV_OFF, ONES_OFF, ONES2_OFF, COLS, blob, miscf, outp, sing, hsb_pool,
          attp, ps, abl="full", pairq=True, has_bias=True):
    QCH = _qchunks(SPL)
    NKCL = (SPL + 127) // 128
    # pairs of consecutive q chunks share one 2-bank psum tile / one exp op
    if pairq:
        PAIRS = [QCH[i:i + 2] for i in range(0, len(QCH), 2)]
    else:
        PAIRS = [[c] for c in QCH]
    PW = max(len(p) for p in PAIRS)
    SBUFS = 3 if PW == 2 else 4

    bsb = sing.tile([128, COLS], MMDT)
    msb = sing.tile([128, NKC + 65], F32)
    KGS = 2  # kc chunks per DMA/emission group
    KG = [(0, min(KGS, NKC))]
    while KG[-1][1] < NKC:
        KG.append((KG[-1][1], min(KG[-1][1] + KGS, NKC)))

    # Input DMA over three queues (SP + ACT HWDGE rings, gpsimd SWDGE).
    # Ordered so the attention front (WK, WQ, ete, early xT groups) lands
    # first and scores/exp can start while the tail streams in.
    def xt_dma(eng, gi):
        k0, k1 = KG[gi]
        c0, c1 = XT_OFF + k0 * 512, XT_OFF + k1 * 512
        eng.dma_start(out=bsb[:, c0:c1], in_=blob[:, c0:c1])

    nc.sync.dma_start(out=bsb[:, WK_OFF:WQ_OFF], in_=blob[:, WK_OFF:WQ_OFF])
    nc.scalar.dma_start(out=bsb[:, WQ_OFF:BK_OFF], in_=blob[:, WQ_OFF:BK_OFF])
    nc.gpsimd.dma_start(out=bsb[:, ETE_OFF:ETE_OFF + SP],
                        in_=blob[:, ETE_OFF:ETE_OFF + SP])
    qs = [nc.sync, nc.scalar, nc.gpsimd]
    for gi in range(len(KG)):
        xt_dma(qs[gi % 3], gi)
    nc.scalar.dma_start(out=bsb[:, :WK_OFF], in_=blob[:, :WK_OFF])  # WV
    nc.gpsimd.dma_start(out=bsb[:, ETE_OFF + SP:], in_=blob[:, ETE_OFF + SP:])
    nc.scalar.dma_start(out=bsb[:, BK_OFF:XT_OFF], in_=blob[:, BK_OFF:XT_OFF])
    nc.sync.dma_start(out=msb, in_=miscf[:, :])

    qk = sing.tile([128, 4, SP], BF if abl == "bf16sc" else MMDT)  # qT 0-1, kT 2-3
    vall = sing.tile([128, NKC, 65 * HPC], BF)
    scr_d = sing.tile([1, 1], MMDT)
    scr_a = sing.tile([1, 1], F32)

    ones_row = bsb[0:1, ONES_OFF:ONES_OFF + SP]

    # Engine preambles: each engine observes the input DMA lanes via a cheap
    # op so no later instruction needs two fresh semaphore waits (hardware
    # allows one sync wait per instruction; extras cost event-sem splits).
    nc.vector.tensor_copy(scr_d, bsb[0:1, 0:1])
    nc.vector.tensor_copy(scr_d, bsb[0:1, XT_OFF:XT_OFF + 1])
    nc.vector.tensor_copy(scr_d, bsb[0:1, ETE_OFF:ETE_OFF + 1])
    nc.scalar.copy(scr_a, msb[0:1, 0:1])

    # ones columns of v_aug, early so PE's DVE clock covers them
    ones2 = bsb[:, ONES2_OFF:ONES2_OFF + HPC]
    for kc in range(NKC):
        va = vall[:, kc, :].rearrange("p (h c) -> p h c", c=65)
        nc.vector.tensor_copy(va[:, :, 64:65], ones2.rearrange("p (h c) -> p h c", c=1))

    if abl == "dmas":
        return

    # ---- projections, interleaved with the progressive xT DMA groups:
    # group g unlocks v[kc in g] and the k/q chunks whose kc range lies in
    # groups <= g. v first, then k; q last (waits on the ete DMA anyway).
    def v_proj(kc):
        pv = ps.tile([128, PW, 512], F32, tag="s2", bufs=SBUFS, name="pv")
        if has_bias:
            nc.tensor.matmul(pv[:, 0, :256], ones_row[:, 0:128], bsb[0:1, BV_OFF:BV_OFF + 256],
                             start=True, stop=False)
        for f in range(4):
            base = XT_OFF + (kc * 4 + f) * 128
            lhsT = bsb[:, base:base + 128]
            rhs = bsb[:, WV_OFF + f * 256:WV_OFF + (f + 1) * 256]
            nc.tensor.matmul(pv[:, 0, :256], lhsT, rhs,
                             start=(f == 0 and not has_bias), stop=(f == 3))
        va = vall[:, kc, :].rearrange("p (h c) -> p h c", c=65)
        nc.vector.tensor_copy(va[:, :, 0:64], pv[:, 0, :256].rearrange("p (h c) -> p h c", c=64))

    def kq_proj(cc, pair):
        p = ps.tile([128, PW, 512], F32, tag="s2", bufs=SBUFS, name="p")
        for j, (qoff, qlen) in enumerate(pair):
            if cc >= 2 and has_bias:  # k: rank-1 bias init (bk x ones)
                bksl = bsb[0:1, BK_OFF + (cc - 2) * 128:BK_OFF + (cc - 1) * 128]
                nc.tensor.matmul(p[:, j, :qlen], bksl, ones_row[:, qoff:qoff + qlen],
                                 start=True, stop=False)
            nparts = len(_xt_moving(bsb, XT_OFF, SP, 0, qoff, qlen))
            for pi in range(nparts):
                for f in range(4):
                    if cc < 2:
                        woff = WQ_OFF + f * 256 + cc * 128
                    else:
                        woff = WK_OFF + f * 256 + (cc - 2) * 128
                    lhsT = bsb[:, woff:woff + 128]
                    loff, llen, ap = _xt_moving(bsb, XT_OFF, SP, f, qoff, qlen)[pi]
                    nc.tensor.matmul(p[:, j, loff:loff + llen], lhsT, ap,
                                     start=(f == 0 and (cc < 2 or not has_bias)),
                                     stop=(f == 3))
        qoff0 = pair[0][0]
        width = (512 + pair[1][1]) if len(pair) == 2 else pair[0][1]
        pview = p[:].rearrange("p a b -> p (a b)")[:, :width]
        if cc < 2:  # q: add etype_emb (includes bq)
            ete_sl = bsb[:, ETE_OFF + cc * SP + qoff0:ETE_OFF + cc * SP + qoff0 + width]
            nc.vector.tensor_add(qk[:, cc, qoff0:qoff0 + width], pview, ete_sl)
        else:
            nc.vector.tensor_copy(qk[:, cc, qoff0:qoff0 + width], pview)

    QCH_K = _qchunks(SP)
    if pairq:
        PAIRS_K = [QCH_K[i:i + 2] for i in range(0, len(QCH_K), 2)]
    else:
        PAIRS_K = [[c] for c in QCH_K]

    def pair_group(pair):  # last xT group this pair's kc range touches
        qoff, qlen = pair[-1]
        return ((qoff + qlen - 1) // 128) // KGS

    for gi, (k0, k1) in enumerate(KG):
        for kc in range(k0, k1):
            v_proj(kc)
        for cc in (2, 3):
            for pair in PAIRS_K:
                if pair_group(pair) == gi:
                    kq_proj(cc, pair)
    for cc in (0, 1):
        for pair in PAIRS:
            kq_proj(cc, pair)

    if abl == "proj":
        return

    # ---- attention, software-pipelined: the scores matmuls for step i+1 are
    # emitted before step i's PV matmuls, so the in-order PE never sits
    # behind the exp running on ACT.
    def scores_mm(step, sp_tile):
        h, ip, kc = step
        cbase = (h % 2) * 64
        for j, (qoff, qlen) in enumerate(PAIRS[ip]):
            lhsT = qk[cbase:cbase + 64, 2 + h // 2, kc * 128:(kc + 1) * 128]
            rhs = qk[cbase:cbase + 64, h // 2, qoff:qoff + qlen]
            nc.tensor.matmul(sp_tile[:, j, :qlen], lhsT, rhs, start=True, stop=True)

    steps = [(h, ip, kc)
             for h in range(HPC) for ip in range(len(PAIRS)) for kc in range(NKC)]
    hts = None
    hp = {}
    DEPTH = 2
    sp_q = []
    for d in range(min(DEPTH, len(steps))):
        t = ps.tile([128, PW, 512], F32, tag="s2", bufs=SBUFS, name="sp_t")
        scores_mm(steps[d], t)
        sp_q.append(t)
    for i, step in enumerate(steps):
        h, ip, kc = step
        pair = PAIRS[ip]
        width = (512 + pair[1][1]) if len(pair) == 2 else pair[0][1]
        sp_cur = sp_q.pop(0)
        if i + DEPTH < len(steps):
            sp_next = ps.tile([128, PW, 512], F32, tag="s2", bufs=SBUFS, name="sp_t")
            scores_mm(steps[i + DEPTH], sp_next)
            sp_q.append(sp_next)
        att = attp.tile([128, PW, 512], BF, tag="att")
        nc.scalar.activation(att[:].rearrange("p a b -> p (a b)")[:, :width],
                             sp_cur[:].rearrange("p a b -> p (a b)")[:, :width],
                             mybir.ActivationFunctionType.Exp,
                             bias=msb[:, kc:kc + 1], scale=0.125)
        if abl != "nopv":
            if kc == 0 and ip == 0:
                hts = hsb_pool.tile([65, NKCL * 128], F32, tag="hts")
            for j, (qoff, qlen) in enumerate(pair):
                if kc == 0:
                    hp[j] = ps.tile([65, 512], F32, tag="h", name="hp")
                nc.tensor.matmul(hp[j][:, :qlen], vall[:, kc, h * 65:(h + 1) * 65],
                                 att[:, j, :qlen], start=(kc == 0), stop=(kc == NKC - 1))
            if kc == NKC - 1:
                for j, (qoff, qlen) in enumerate(pair):
                    nc.vector.tensor_copy(hts[:, qoff:qoff + qlen], hp[j][:, :qlen])
                if ip == len(PAIRS) - 1:  # head done: ship hT (+den row); the
                    # host does the [64, q] -> [q, 64] transpose and divide
                    nc.sync.dma_start(out=outp[h, :, :NKCL * 128], in_=hts[:])




def _prep_core(core, SP, x, etype_emb, mask, Wq, bq, Wk, bk, Wv, bv):
    NKC = SP // 128
    (WV_OFF, WK_OFF, WQ_OFF, XT_OFF, ETE_OFF, BK_OFF, BV_OFF, ONES_OFF,
     ONES2_OFF, COLS) = _offsets(SP)
    b, hg = core // 2, core % 2
    c0 = hg * CPC
    idx = np.where(mask[b] == 1)[0]
    Su = len(idx)

    blob = np.zeros((128, COLS), ml_dtypes.bfloat16)
    xs = np.zeros((SP, F), np.float32)
    xs[:Su] = x[b][idx]
    xT = xs.T
    xtb = xT.reshape(4, 128, NKC, 128).transpose(1, 2, 0, 3).reshape(128, NKC * 512)
    blob[:, XT_OFF:XT_OFF + 4 * SP] = xtb
    for f in range(4):
        blob[:, WV_OFF + f * 256:WV_OFF + (f + 1) * 256] = Wv[f * 128:(f + 1) * 128, c0:c0 + CPC]
        blob[:, WK_OFF + f * 256:WK_OFF + (f + 1) * 256] = Wk[f * 128:(f + 1) * 128, c0:c0 + CPC]
        blob[:, WQ_OFF + f * 256:WQ_OFF + (f + 1) * 256] = Wq[f * 128:(f + 1) * 128, c0:c0 + CPC]
    et = np.zeros((SP, CPC), np.float32)
    et[:Su] = etype_emb[b][idx][:, c0:c0 + CPC] + bq[c0:c0 + CPC]
    etT = et.T
    blob[:, ETE_OFF:ETE_OFF + SP] = etT[:128]
    blob[:, ETE_OFF + SP:ETE_OFF + 2 * SP] = etT[128:]
    blob[0, BK_OFF:BK_OFF + CPC] = bk[c0:c0 + CPC]
    blob[0, BV_OFF:BV_OFF + CPC] = bv[c0:c0 + CPC]
    blob[0, ONES_OFF:ONES_OFF + SP] = 1.0
    blob[:, ONES2_OFF:ONES2_OFF + HPC] = 1.0

    miscf = np.zeros((128, NKC + 65), np.float32)
    pos = np.arange(128)[:, None] + 128 * np.arange(NKC)[None, :]
    miscf[:, :NKC] = np.where(pos < Su, 0.0, -30000.0)
    miscf[:65, NKC:NKC + 65] = np.eye(65, dtype=np.float32)

    return {"blob": blob, "miscf": miscf}, idx


def kernel(x, etype_emb, mask, Wq, bq, Wk, bk, Wv, bv):
    global LAST_RESULT
    x = np.asarray(x, np.float32)
    etype_emb = np.asarray(etype_emb, np.float32)
    mask = np.asarray(mask)
    Wq, bq = np.asarray(Wq, np.float32), np.asarray(bq, np.float32)
    Wk, bk = np.asarray(Wk, np.float32), np.asarray(bk, np.float32)
    Wv, bv = np.asarray(Wv, np.float32), np.asarray(bv, np.float32)

    counts = [int((mask[b] == 1).sum()) for b in range(B)]
    SPL = max(2, max(counts))
    SPL += SPL % 2  # fp32r matmuls with 128 contraction rows need even N
    SP = max(128, ((SPL + 127) // 128) * 128)

    has_bias = bool(np.any(bk) or np.any(bv))
    nc = _build(SP, SPL=SPL, has_bias=has_bias)
    in_maps, idxs = [], []
    for core in range(NCORES):
        m, idx = _prep_core(core, SP, x, etype_emb, mask, Wq, bq, Wk, bk, Wv, bv)
        in_maps.append(m)
        idxs.append(idx)

    # The NTFF trace path needs antenv.axon_hooks, which this container does
    # not ship; make sure a stray BASS_TRACE=1 cannot route us into it.
    os.environ.setdefault("BASS_NEVER_TRACE", "1")
    res = run_bass_kernel_spmd(nc, in_maps, list(range(NCORES)))
    LAST_RESULT = res

    out = np.zeros((B, S, E), np.float32)
    for core in range(NCORES):
        b, hg = core // 2, core % 2
        idx = idxs[core]
        if not len(idx):
            continue
        shard = res.results[core]["out"]  # [HPC, 65, SP]: hT rows + denominator
        for h in range(HPC):
            num = shard[h, :64, :len(idx)]
            den = shard[h, 64, :len(idx)]
            out[b][idx, hg * CPC + h * 64:hg * CPC + (h + 1) * 64] = (num / den).T
    return out

